# revision 1
# baseline (speedup 1.0000x reference)
"""Trainium2 kernel for nn_LocalSorterModel (gnn_message_passing).

The reference model is entirely linear (pair-gather -> linear -> reshape ->
linear, no nonlinearity), so the whole network collapses exactly into a
single affine map:

    out[b, r] = sum_{n,d} embeds[b, n, d] * M[r, n*D + d] + const[r]

with M [120, 5120] / const [120] precomputed on the host from the small
weights (see _collapse_weights).

Device work: a [2048, 5120] @ [5120, 120] matmul, memory-bound. Sharding:
4 contraction slices x 2 batch halves across 8 cores (pure SPMD).

Per-core pipeline:
  - X is quantized host-side to fp8 e3m4 with one scale per contraction
    row, folded exactly into the fp16 weights (W'[k,r] = W[k,r]*s_k). The
    PE consumes the fp8 moving operand directly against the fp16
    stationary operand - no on-chip dtype conversion stage at all.
    HW-measured rel err 9.8e-3 vs the 2e-2 gate (fp16 everywhere: 3e-4).
  - X streams in batch slices, interleaved across both HWDGE issue
    engines; W streams in two pieces so the first matmuls only wait for
    the first chunk group. A few junk warmup matmuls keep the PE p-state
    ramped during DMA fill.
  - Each slice accumulates in its own PSUM bank, so its PSUM->SBUF copy
    (DVE) fires as soon as that slice's accumulation stops; outputs leave
    as two DMAs on separate sequencers (bulk early, small tail last).
  - An alternative SWDGE prepared-scatter/trigger output path
    (OUT_SCATTER=True) models ~1.3us faster but aborts the NEFF on real
    hardware, so it stays disabled.
Host gathers: out = sum_kf partial[kf].T + const.
"""

import os

os.environ.setdefault("NEURON_RT_RESET_CORES", "1")

import numpy as np
import ml_dtypes

import concourse.bacc as bacc
import concourse.mybir as mybir
from concourse.tile import TileContext
from concourse.bass_utils import run_bass_kernel_spmd

B = 2048          # batch
NI = 5            # items
D = 1024          # embed dim
KT = NI * D       # 5120 total contraction
R = 120           # num results
KF = 4            # contraction shards
BF = 2            # batch shards
NCORES = KF * BF  # 8
KC = KT // KF     # 1280 contraction per core
NKC = KC // 128   # 10 chunks of 128
BL = B // BF      # 1024 batch per core

# Batch-slice widths per core (must group into 512-col PSUM banks).
SLICES = [256, 240, 256, 208, 64]
assert sum(SLICES) == BL
_OFF = np.concatenate([[0], np.cumsum(SLICES)])  # col offsets

WSPLIT = 6        # w streams as chunks [0,WSPLIT) then [WSPLIT,NKC)
NWARM = 6 
OUT_SCATTER = False  # SWDGE prep/trigger outputs crash the NEFF on HW; plain HWDGE DMAs

_f16 = mybir.dt.float16
_f32 = mybir.dt.float32
_f8 = mybir.dt.float8e3
_i16 = mybir.dt.int16
_np_f8 = ml_dtypes.float8_e3m4
F8MAX = 15.5

_CACHE = {}


def _build_nc():
    nc = bacc.Bacc("TRN2", target_bir_lowering=False, debug=False,
                   num_swdge_queues=2)
    x8 = nc.dram_tensor("x8", [128, NKC * BL], _f8, kind="ExternalInput")
    w = nc.dram_tensor("w", [128, NKC * R + 8], _f16, kind="ExternalInput")
    o = nc.dram_tensor("o", [R, BL], _f16, kind="ExternalOutput")

    nsl = len(SLICES)
    with TileContext(nc) as tc:
        with (
            tc.tile_pool(name="sb", bufs=1) as sb,
            tc.tile_pool(name="pp", bufs=1, space="PSUM") as pp,
        ):
            # --- junk tiles for PE warmup -------------------------------
            jw = sb.tile([128, 128], _f16, tag="jw", name="jw")
            jx = sb.tile([128, 512], _f16, tag="jx", name="jx")
            pj = pp.tile([128, 512], _f32, tag="pj", name="pj")
            nc.vector.memset(jw[:], 0.0)
            nc.vector.memset(jx[:], 0.0)
            for _ in range(NWARM):
                nc.tensor.matmul(
                    pj[:, :], jw[:], jx[:], start=True, stop=True,
                    skip_group_check=True,
                )

            # --- input tiles + interleaved DMA stream -------------------
            xt = [
                sb.tile([128, NKC * wd], _f8, tag=f"x{s}", name=f"x{s}")
                for s, wd in enumerate(SLICES)
            ]
            wt = sb.tile([128, NKC * R + 8], _f16, tag="w", name="wt")
            # scatter row-index table rides in the last 8 f16 columns of W;
            # staged into an aligned dedicated tile for the Q7 desc-gen
            idxt = sb.tile([16, 8], _i16, tag="idx", name="idxt")

            hs = min(WSPLIT, NKC) * R
            nc.sync.dma_start(xt[0][:], x8[:, 0 : NKC * _OFF[1]])        # x0
            nc.scalar.dma_start(wt[:, 0:hs], w[:, 0:hs])                  # wa
            nc.sync.dma_start(wt[:, hs:], w[:, hs:])                      # wb
            for s in range(1, nsl):
                eng = nc.scalar if s % 2 else nc.sync
                eng.dma_start(
                    xt[s][:], x8[:, NKC * _OFF[s] : NKC * _OFF[s + 1]]
                )

            # --- matmuls: one PSUM bank per slice -----------------------
            ps = [
                pp.tile([R, wd], _f32, tag=f"ps{s}", name=f"ps{s}")
                for s, wd in enumerate(SLICES)
            ]

            def mms(s, c0, c1):
                wd = SLICES[s]
                for c in range(c0, c1):
                    nc.tensor.matmul(
                        ps[s][:, :],
                        wt[:, c * R : (c + 1) * R],
                        xt[s][:, c * wd : (c + 1) * wd],
                        start=(c == 0),
                        stop=(c == NKC - 1),
                        skip_group_check=True,
                    )

            ws = min(WSPLIT, NKC)
            mms(0, 0, ws)
            mms(0, ws, NKC)
            for s in range(1, nsl):
                mms(s, 0, NKC)

            # --- per-slice copy (DVE) + scatter-DMA outputs -------------
            # Output DMAs leave via SWDGE scatters whose descriptors are
            # prepared early on GPSIMD (no data deps); the data-dependent
            # trigger is nearly free, so the tail skips the ~1.3us HWDGE
            # issue chain. oA covers all but the last slice; oB is the
            # last slice's small piece.
            # otA = all but the last two slices (fires mid-kernel, fully
            # overlapped); otB = the last two slices (small tail piece).
            nA = nsl - 2
            wA = int(_OFF[nA])
            wB = BL - wA
            otA = sb.tile([128, wA], _f16, tag="otA", name="otA")
            otB = sb.tile([128, wB], _f16, tag="otB", name="otB")
            nc.vector.memset(otA[:], 0.0)
            nc.vector.memset(otB[:], 0.0)
            for s in range(nA):
                nc.vector.tensor_copy(
                    otA[0:R, _OFF[s] : _OFF[s + 1]], ps[s][:, :]
                )
            for s in range(nA, nsl):
                nc.vector.tensor_copy(
                    otB[0:R, _OFF[s] - wA : _OFF[s + 1] - wA], ps[s][:, :]
                )
            if OUT_SCATTER:
                nc.vector.tensor_copy(
                    idxt[:], wt[0:16, NKC * R : NKC * R + 8].bitcast(_i16)
                )
                # Both preps dispatch before either trigger so descriptor
                # gen runs early; separate queues keep each trigger bound
                # to its own prep's deferred data deps.
                prep_a = nc.gpsimd.dma_scatter_add(
                    o[:, 0:wA], otA[:].unsqueeze(1), idxt[:], 128, R, wA,
                    elem_step=BL, prepare_only=True, queue_num=0,
                    single_packet=False,
                    sem=nc.alloc_semaphore("oscatA"),
                )
                prep_b = nc.gpsimd.dma_scatter_add(
                    o[:, wA:BL], otB[:].unsqueeze(1), idxt[:], 128, R, wB,
                    elem_step=BL, prepare_only=True, queue_num=1,
                    single_packet=False,
                    sem=nc.alloc_semaphore("oscatB"),
                )
                trig_a = nc.gpsimd.trigger_dma(count=None, queue_num=0)
                nc.gpsimd.trigger_dma(count=None, queue_num=1)
                from concourse.tile import add_dep_helper
                add_dep_helper(trig_a.ins, prep_b.ins, False,
                               "prep B desc-gen before trig A holds SEQ")
            else:
                nc.scalar.dma_start(o[:, 0:wA], otA[0:R, :])
                nc.sync.dma_start(o[:, wA:BL], otB[0:R, :])
    nc.compile()
    if OUT_SCATTER:
        _patch_prep_sems(nc)
    return nc


def _patch_prep_sems(nc):
    """Route prepared-scatter completion to Tile's DMASW lane semaphores.

    Tile assigns each SWDGE prep a DMASW proc lane and the teardown drain
    waits for that lane's semaphore to reach 16 per DMA -- but the sem
    baked into the descriptors is the user-supplied `sem=`, and Tile never
    attaches the lane-sem increment anywhere (prepared DMAs are not
    exercised under Tile upstream). Rewriting each prep's on_update[0]
    (what walrus bakes into the descriptor, and what the cost model fires
    at trigger time) to the lane sem closes the loop for both the
    simulator and hardware.
    """
    fn = nc.m.functions[0]
    insts = [i for blk in fn.blocks for i in blk.instructions]
    lanes = {}
    for inst in insts:
        si = inst.sync_info
        if not si:
            continue
        for wt_ in si.on_wait:
            if wt_.ant_name and wt_.ant_name.startswith("DMASW"):
                lanes[wt_.ant_name] = wt_
    lane_waits = [lanes[k] for k in sorted(lanes)]
    preps = [
        i for i in insts
        if type(i).__name__ == "InstDMAScatterAddAnt"
        and getattr(i, "gen_mode", 0) == 1
    ]
    assert len(preps) == len(lane_waits), (len(preps), len(lane_waits))
    for prep, lw in zip(preps, lane_waits):
        si = prep.sync_info
        ups = list(si.on_update)
        old = ups[0]
        ups[0] = mybir.SyncUpdate(
            sync_type=old.sync_type, id=lw.id, ant_name=lw.ant_name,
            update_mode=old.update_mode, update_value=old.update_value,
            update_reg=old.update_reg,
        )
        si.on_update = ups
    nc.compile()
    return nc


def _collapse_weights(pw_w, pw_b, cls_w, cls_b):
    """Exact linearization of the model -> (M_T [5120, 120] f32, const [120] f32)."""
    mask = ~np.eye(NI, dtype=bool)
    idx_i, idx_j = np.nonzero(mask)  # 20 ordered off-diagonal pairs, row-major

    cw = cls_w.reshape(R, NI * (NI - 1), D).astype(np.float64)
    w3 = np.zeros((R, NI, D))
    w4 = np.zeros((R, NI, D))
    for p in range(NI * (NI - 1)):
        w3[:, idx_i[p], :] += cw[:, p, :]
        w4[:, idx_j[p], :] += cw[:, p, :]
    w1 = pw_w[:, :D].astype(np.float64)
    w2 = pw_w[:, D:].astype(np.float64)
    m = w3.reshape(R * NI, D) @ w1 + w4.reshape(R * NI, D) @ w2  # [600, 1024]
    m = m.reshape(R, KT)
    const = cw.sum(axis=1) @ pw_b.astype(np.float64) + cls_b
    m_t = np.ascontiguousarray(m.T).astype(np.float32)  # [5120, 120]
    return m_t, const.astype(np.float32)


def prepare_in_maps(embeds, pw_w, pw_b, cls_w, cls_b):
    """Host-side prep: collapse weights, fp8-quantize X, pack per-core inputs.

    Returns (in_maps, const).
    """
    embeds = np.asarray(embeds, dtype=np.float32)
    m_t, const = _collapse_weights(
        np.asarray(pw_w, np.float32), np.asarray(pw_b, np.float32),
        np.asarray(cls_w, np.float32), np.asarray(cls_b, np.float32),
    )

    xf = embeds.reshape(B, KT)
    # fp8 e3m4 with one scale per contraction row, folded into W exactly.
    s_k = np.abs(xf).max(axis=0) / F8MAX  # [KT]
    np.maximum(s_k, 1e-12, out=s_k)
    xq = (xf / s_k[None, :]).astype(_np_f8)  # [B, KT] fp8
    wp = (m_t * s_k[:, None]).astype(np.float16)  # [KT, R]

    j = np.arange(128)
    vals = np.where(j < R, j, -1).astype(np.int16)
    idx_np = np.ascontiguousarray(vals.reshape(8, 16).T)  # idx[p, f] = 16f+p
    in_maps = []
    for core in range(NCORES):
        kf, bf = divmod(core, BF)
        t = np.ascontiguousarray(
            xq[bf * BL : (bf + 1) * BL, kf * KC : (kf + 1) * KC].T
        )  # [KC, BL] fp8
        # x8[p, slice_off*NKC + c*wd + col] = t[c*128 + p, _OFF[s] + col]
        x_c = np.empty((128, NKC * BL), dtype=_np_f8)
        t3 = t.reshape(NKC, 128, BL)
        for s, wd in enumerate(SLICES):
            blk = t3[:, :, _OFF[s] : _OFF[s + 1]]  # [NKC, 128, wd]
            x_c[:, NKC * _OFF[s] : NKC * _OFF[s + 1]] = (
                blk.transpose(1, 0, 2).reshape(128, NKC * wd)
            )
        # w packed [128, NKC*R]: w[p, c*R + r] = wp[kf*KC + c*128 + p, r]
        # plus 8 trailing f16 columns carrying the int16 scatter indices.
        w_c = np.zeros((128, NKC * R + 8), dtype=np.float16)
        w_c[:, : NKC * R] = (
            wp[kf * KC : (kf + 1) * KC, :]
            .reshape(NKC, 128, R)
            .transpose(1, 0, 2)
            .reshape(128, NKC * R)
        )
        idx16 = np.zeros((128, 8), dtype=np.int16)
        idx16[:16] = idx_np
        w_c[:, NKC * R :] = idx16.view(np.float16)
        in_maps.append({"x8": x_c, "w": w_c})
    return in_maps, const


def kernel(embeds, pw_w, pw_b, cls_w, cls_b):
    in_maps, const = prepare_in_maps(embeds, pw_w, pw_b, cls_w, cls_b)

    if "nc" not in _CACHE:
        _CACHE["nc"] = _build_nc()
    res = run_bass_kernel_spmd(_CACHE["nc"], in_maps, core_ids=list(range(NCORES)))

    out = np.empty((B, R), dtype=np.float32)
    for bf in range(BF):
        acc = np.zeros((R, BL), dtype=np.float32)
        for kf in range(KF):
            acc += res.results[kf * BF + bf]["o"].astype(np.float32)
        out[bf * BL : (bf + 1) * BL, :] = acc.T
    out += const[None, :]
    return out



# revision 7
# speedup vs baseline: 1.0632x; 1.0632x over previous
"""Trainium2 kernel for nn_LocalSorterModel (gnn_message_passing).

The reference model is entirely linear (pair-gather -> linear -> reshape ->
linear, no nonlinearity), so the whole network collapses exactly into a
single affine map:

    out[b, r] = sum_{n,d} embeds[b, n, d] * M[r, n*D + d] + const[r]

with M [120, 5120] / const [120] precomputed on the host from the small
weights (see _collapse_weights).

Device work: a [2048, 5120] @ [5120, 120] matmul, memory-bound. Sharding:
4 contraction slices x 2 batch halves across 8 cores (pure SPMD).

Per-core pipeline (v2.1):
  - X is quantized host-side to fp8, one scale per contraction row, folded
    exactly into the weights. Chunks 0-3 of each core's 10 contraction
    chunks are fp8 e4m3 and run as DoubleRow matmuls (2 chunks / matmul at
    0.5 cyc/row) against e4m3 weights stored as an exact hi+lo pair (the
    lo term cancels the e4m3 weight quantization error); chunks 4-9 stay
    e3m4 against fp16 weights. Measured rel err ~1.5e-2 vs the 2e-2 gate.
  - All input DMAs stream back-to-back on the SP queue: x-slice 0 (which
    also carries the fp8 DoubleRow weight bytes), the fp16 weights, then
    the remaining x slices, sized so neither the DMA engines nor the PE
    ever fall behind. Junk warmup/filler matmuls hold the PE p-state ramp.
  - Each compute slice accumulates in its own PSUM bank (the last x slice
    is split into two small banks so the final copy is tiny); DVE copies
    banks to SBUF fp16 as they stop, the final tiny copy runs on the idle
    Activation engine. Outputs leave as two DMAs on the SP queue: slices
    0-3 early, the small tail late.
Host gathers: out[bf] = sum_kf t_kf * concat(oA, oB).T, + const.
"""

import os

os.environ.setdefault("NEURON_RT_RESET_CORES", "1")

import numpy as np
import ml_dtypes

import concourse.bacc as bacc
import concourse.mybir as mybir
from concourse.tile import TileContext
from concourse.bass_utils import run_bass_kernel_spmd

B = 2048          # batch
NI = 5            # items
D = 1024          # embed dim
KT = NI * D       # 5120 total contraction
R = 120           # num results
KF = 4            # contraction shards
BF = 2            # batch shards
NCORES = KF * BF  # 8
KC = KT // KF     # 1280 contraction per core
NKC = KC // 128   # 10 chunks of 128
BL = B // BF      # 1024 batch per core

NDR = 4           # chunks 0..NDR-1 are e4m3 DoubleRow (must be even)
NPAIR = NDR // 2
NE3 = NKC - NDR   # trailing e3m4 chunks

# X DMA slice widths (transfer stream), and compute sub-slices (PSUM
# groups). The last DMA slice is split into two PSUM groups so the final
# copy is tiny. 8 PSUM banks: junk + 5 + 2.
SLICES = [128, 192, 192, 192, 176, 144]
assert sum(SLICES) == BL
_OFF = np.concatenate([[0], np.cumsum(SLICES)])
NSL = len(SLICES)
LAST_SPLIT = 64   # width of the final PSUM group (tail of the last slice)
# compute groups: (dma_slice, col_lo, col_hi) in slice-local cols
GROUPS = [(s, 0, SLICES[s]) for s in range(NSL - 1)]
GROUPS += [
    (NSL - 1, 0, SLICES[-1] - LAST_SPLIT),
    (NSL - 1, SLICES[-1] - LAST_SPLIT, SLICES[-1]),
]
NG = len(GROUPS)
NG_A = 4          # first NG_A groups -> oA, rest -> oB
WA = int(_OFF[NG_A])   # groups 0..3 are whole slices 0..3
WB = BL - WA

RW = 128                  # stationary cols, padded from R=120 (dual-fp8
                          # Ldweights requires exactly 128)
WDR = NPAIR * 4 * RW      # fp8 W bytes (hi+lo per pair), rides in x0 DMA
X0B = NKC * SLICES[0] + WDR
W16C = NE3 * RW           # fp16 W columns (padded)

NWARM = 22        # 128-wide junk matmuls before slice 0
FILLERS = [10, 1, 0, 0, 0, 0, 0]   # junk matmuls after each group (ramp keep-alive)

_f16 = mybir.dt.float16
_f32 = mybir.dt.float32
_f8e3 = mybir.dt.float8e3
_f8e4 = mybir.dt.float8e4
_np_e3 = ml_dtypes.float8_e3m4
_np_e4 = ml_dtypes.float8_e4m3   # TRN fp8e4: max normal 240
E3MAX = 15.5
E4MAX = 240.0
_DR = mybir.MatmulPerfMode.DoubleRow

_CACHE = {}


def _build_nc():
    nc = bacc.Bacc("TRN2", target_bir_lowering=False, debug=False)
    xts_d = []
    for s, wd in enumerate(SLICES):
        nb = X0B if s == 0 else NKC * wd
        xts_d.append(nc.dram_tensor(f"x{s}", [128, nb], _f8e3, kind="ExternalInput"))
    w16_d = nc.dram_tensor("w16", [128, W16C], _f16, kind="ExternalInput")
    oa_d = nc.dram_tensor("oA", [R, WA], _f16, kind="ExternalOutput")
    ob_d = nc.dram_tensor("oB", [R, WB], _f16, kind="ExternalOutput")

    with TileContext(nc) as tc:
        with (
            tc.tile_pool(name="sb", bufs=1) as sb,
            tc.tile_pool(name="pp", bufs=1, space="PSUM") as pp,
        ):
            # --- junk tile + warmup matmuls (PE p-state ramp) -----------
            jw = sb.tile([128, 128], _f16, tag="jw", name="jw")
            pj = pp.tile([128, 128], _f32, tag="pj", name="pj")
            nc.gpsimd.memset(jw[:], 0.0)

            def junk(n):
                for _ in range(n):
                    nc.tensor.matmul(
                        pj[:, :], jw[:], jw[:], start=True, stop=True,
                        skip_group_check=True,
                    )

            junk(NWARM)

            # --- input tiles, all DMAs on the SP queue ------------------
            xt = [
                sb.tile([128, X0B if s == 0 else NKC * wd], _f8e3,
                        tag=f"x{s}", name=f"x{s}")
                for s, wd in enumerate(SLICES)
            ]
            wt = sb.tile([128, W16C], _f16, tag="w16", name="wt")

            nc.sync.dma_start(xt[0][:], xts_d[0][:])
            nc.sync.dma_start(wt[:], w16_d[:])
            for s in range(1, NSL):
                nc.sync.dma_start(xt[s][:], xts_d[s][:])

            # --- weight views -------------------------------------------
            wdr0 = NKC * SLICES[0]  # fp8 W bytes start inside x0 tile

            def w_pair(i, lo):
                off = wdr0 + (2 * i + lo) * 2 * RW
                return (
                    xt[0][:, off : off + 2 * RW]
                    .bitcast(_f8e4)
                    .rearrange("p (k r) -> p k r", k=2)
                )

            # --- matmuls: one PSUM bank per compute group ---------------
            ps = [
                pp.tile([128, hi - lo], _f32, tag=f"ps{g}", name=f"ps{g}")
                for g, (s, lo, hi) in enumerate(GROUPS)
            ]

            for g, (s, lo, hi) in enumerate(GROUPS):
                wd = SLICES[s]
                gw = hi - lo

                first = True
                for i in range(NPAIR):
                    # [128, 2, gw] e4m3 view of cols [lo,hi) of the chunk
                    # pair (2i, 2i+1)
                    xp = (
                        xt[s][:, (2 * i) * wd : (2 * i + 2) * wd]
                        .bitcast(_f8e4)
                        .rearrange("p (k w) -> p k w", k=2)[:, :, lo:hi]
                    )
                    for l in (0, 1):
                        nc.tensor.matmul(
                            ps[g][:, :], w_pair(i, l), xp,
                            start=first, stop=False, perf_mode=_DR,
                            skip_group_check=True,
                        )
                        first = False
                for c in range(NDR, NKC):
                    nc.tensor.matmul(
                        ps[g][:, :], wt[:, (c - NDR) * RW : (c - NDR + 1) * RW],
                        xt[s][:, c * wd + lo : c * wd + hi],
                        start=False, stop=(c == NKC - 1),
                        skip_group_check=True,
                    )
                junk(FILLERS[g] if g < len(FILLERS) else 0)

            # --- PSUM->SBUF copies + output DMAs ------------------------
            ota = sb.tile([128, WA], _f16, tag="otA", name="ota")
            otb = sb.tile([128, WB], _f16, tag="otB", name="otb")

            def gseg(g):
                s, lo, hi = GROUPS[g]
                a = int(_OFF[s]) + lo
                return a, a + (hi - lo)

            for g in range(NG - 1):  # DVE: all but the final tiny group
                a, b = gseg(g)
                dst = ota if g < NG_A else otb
                off = 0 if g < NG_A else WA
                nc.vector.tensor_copy(dst[0:R, a - off : b - off], ps[g][0:R, :])
            a, b = gseg(NG - 1)      # Activation: final tiny copy
            nc.scalar.copy(otb[0:R, a - WA : b - WA], ps[NG - 1][0:R, :])

            nc.sync.dma_start(oa_d[:, :], ota[0:R, :])
            nc.sync.dma_start(ob_d[:, :], otb[0:R, :])
    nc.compile()
    return nc


def _collapse_weights(pw_w, pw_b, cls_w, cls_b):
    """Exact linearization of the model -> (M_T [5120, 120] f64, const [120] f32)."""
    mask = ~np.eye(NI, dtype=bool)
    idx_i, idx_j = np.nonzero(mask)  # 20 ordered off-diagonal pairs, row-major

    cw = cls_w.reshape(R, NI * (NI - 1), D).astype(np.float64)
    w3 = np.zeros((R, NI, D))
    w4 = np.zeros((R, NI, D))
    for p in range(NI * (NI - 1)):
        w3[:, idx_i[p], :] += cw[:, p, :]
        w4[:, idx_j[p], :] += cw[:, p, :]
    w1 = pw_w[:, :D].astype(np.float64)
    w2 = pw_w[:, D:].astype(np.float64)
    m = w3.reshape(R * NI, D) @ w1 + w4.reshape(R * NI, D) @ w2  # [600, 1024]
    m = m.reshape(R, KT)
    const = cw.sum(axis=1) @ pw_b.astype(np.float64) + cls_b
    m_t = np.ascontiguousarray(m.T)  # [5120, 120] f64
    return m_t, const.astype(np.float32)


def prepare_in_maps(embeds, pw_w, pw_b, cls_w, cls_b):
    """Host prep: collapse weights, mixed fp8 quantize X, pack per-core inputs.

    Returns (in_maps, (const, t_scales)).
    """
    embeds = np.asarray(embeds, dtype=np.float32)
    m_t, const = _collapse_weights(
        np.asarray(pw_w, np.float32), np.asarray(pw_b, np.float32),
        np.asarray(cls_w, np.float32), np.asarray(cls_b, np.float32),
    )

    xf = embeds.reshape(B, KT)
    # per-contraction-row scales; DR rows (chunks 0..NDR-1 of each kf shard)
    # quantize to e4m3, the rest to e3m4.
    dr_mask = np.zeros(KT, dtype=bool)
    for kf in range(KF):
        dr_mask[kf * KC : kf * KC + NDR * 128] = True
    amax = np.abs(xf).max(axis=0)
    s_k = np.where(dr_mask, amax / E4MAX, amax / E3MAX)
    np.maximum(s_k, 1e-12, out=s_k)
    xs = xf / s_k[None, :]
    xb = np.empty((B, KT), dtype=np.uint8)
    xb[:, dr_mask] = xs[:, dr_mask].astype(_np_e4).view(np.uint8)
    xb[:, ~dr_mask] = xs[:, ~dr_mask].astype(_np_e3).view(np.uint8)

    wf = m_t * s_k[:, None]  # [KT, R] f64, all X scales folded

    in_maps = []
    t_scales = []
    for core in range(NCORES):
        kf, bf = divmod(core, BF)
        # ---- weights for this kf shard, one global scale t ------------
        wk = wf[kf * KC : (kf + 1) * KC]  # [KC, R]
        # t must keep (a) the e4m3 hi weights within +-240 and (b) the
        # worst-case PSUM magnitude within fp16 range for the output copy.
        qmax = np.concatenate(
            [np.full(NDR * 128, E4MAX), np.full(NE3 * 128, E3MAX)]
        )
        s_bound = (qmax[:, None] * np.abs(wk)).sum(axis=0).max()
        t = max(np.abs(wk[: NDR * 128]).max() / E4MAX, s_bound / 60000.0, 1e-12)
        wkt = (wk / t).astype(np.float32)
        hi = np.zeros((NDR * 128, RW), dtype=_np_e4)
        lo = np.zeros((NDR * 128, RW), dtype=_np_e4)
        hi[:, :R] = wkt[: NDR * 128].astype(_np_e4)
        lo[:, :R] = (wkt[: NDR * 128] - hi[:, :R].astype(np.float32)).astype(_np_e4)
        # wdr bytes [128, WDR]: per pair i: hi(chunk 2i | 2i+1), lo(...)
        hi3 = hi.view(np.uint8).reshape(NDR, 128, RW)
        lo3 = lo.view(np.uint8).reshape(NDR, 128, RW)
        wdr = np.empty((128, WDR), dtype=np.uint8)
        for i in range(NPAIR):
            base = i * 4 * RW
            wdr[:, base : base + 2 * RW] = (
                hi3[2 * i : 2 * i + 2].transpose(1, 0, 2).reshape(128, 2 * RW)
            )
            wdr[:, base + 2 * RW : base + 4 * RW] = (
                lo3[2 * i : 2 * i + 2].transpose(1, 0, 2).reshape(128, 2 * RW)
            )
        w16p = np.zeros((NE3 * 128, RW), dtype=np.float16)
        w16p[:, :R] = wkt[NDR * 128 :].astype(np.float16)
        w16 = np.ascontiguousarray(
            w16p.reshape(NE3, 128, RW).transpose(1, 0, 2).reshape(128, W16C)
        )
        t_scales.append(np.float32(t))

        # ---- X slices --------------------------------------------------
        blk = np.ascontiguousarray(
            xb[bf * BL : (bf + 1) * BL, kf * KC : (kf + 1) * KC].T
        )  # [KC, BL] u8
        t3 = blk.reshape(NKC, 128, BL)
        im = {}
        for s, wd in enumerate(SLICES):
            xs_b = (
                t3[:, :, _OFF[s] : _OFF[s + 1]]
                .transpose(1, 0, 2)
                .reshape(128, NKC * wd)
            )
            if s == 0:
                xs_b = np.concatenate([xs_b, wdr], axis=1)
            im[f"x{s}"] = np.ascontiguousarray(xs_b).view(_np_e3)
        im["w16"] = w16
        in_maps.append(im)
    return in_maps, (const, t_scales)


def kernel(embeds, pw_w, pw_b, cls_w, cls_b):
    in_maps, (const, t_scales) = prepare_in_maps(embeds, pw_w, pw_b, cls_w, cls_b)

    if "nc" not in _CACHE:
        _CACHE["nc"] = _build_nc()
    res = run_bass_kernel_spmd(_CACHE["nc"], in_maps, core_ids=list(range(NCORES)))

    out = np.empty((B, R), dtype=np.float32)
    for bf in range(BF):
        acc = np.zeros((R, BL), dtype=np.float32)
        for kf in range(KF):
            core = kf * BF + bf
            r = res.results[core]
            part = np.concatenate(
                [r["oA"].astype(np.float32), r["oB"].astype(np.float32)], axis=1
            )
            acc += t_scales[core] * part
        out[bf * BL : (bf + 1) * BL, :] = acc.T
    out += const[None, :]
    return out
